# revision 1
# baseline (speedup 1.0000x reference)
"""Trainium2 Bass kernel for nn_BatchFlipLoss (NCE batch-flip loss + CE loss).

Math reformulation (validated to rel-err ~1e-7 vs the jax reference):

The reference sums BatchCriterion over 36 flip-class pairs (i,j), j>=i.
For pair (i,j) with x = [f_i; f_j] (f_c = features[c::8], L2-normalized,
B=512 rows each), T=0.1, the loss decomposes over ordered halves (a,b)
(rows of f_a, partner f_b). With E_ab = exp(10*G_ab), G_ab = f_a@f_b.T,
S_ab = rowsum(E_ab), S2_ab = rowsum(E_ab^2), d_ab[p] = f_a[p].f_b[p]:

  D_ab  = S0_aa + S_ab      (S0_aa: diag-zeroed; (a,a): D = 2*S0_aa+e^10)
  half  = 10*d - ln(D) - [N1*R + N2*R^2/2] - ln(1 - exp(10 d)*R)
          where R = 1/D, N_k = Sk0_aa + Sk_ab   (ln(1-x) ~ -(x+x^2/2);
          the x^3 tail is ~1e-6 relative after the alpha/1024 scaling)
  (a,a) pair = 2*(10*d - lnD - [N1*R + N2*R^2/2]), N_k = 2*Sk0_aa
          (the cross-diag term cancels -ln(1-pmt) exactly)

Work sharing: only the 36 unordered blocks are exponentiated. Core c
computes blocks (c, c+j mod 8) for j=0..4 (every unordered pair {a,b}
appears: distance k<=4 on core a, else distance 8-k on core b; the four
distance-4 pairs are computed twice, once per endpoint). Each block
yields BOTH directions' sums:
  rowsum  S_{c,c+j}   : ScalarE Exp accum_out / VectorE stt accum_out
  colsum  S_{c+j,c}   : PE matmul with a ones lhsT, accumulated over the
                        four row-chunks in a [1,512] PSUM bank (only
                        needed for j in {1,2,3}; distance-4 halves use
                        each endpoint's own rowsum)
The host reroutes these O(rows) vectors between cores and applies the
closed-form combine; CE rowsums (exp-accum + iota/is_equal gather) ride
along. All O(N^2) work (matmuls, exp, squares) stays on device.

SPMD: one NEFF for all cores, written for canonical class 0; the host
rotates each core's feature inputs so its own class is block 0 and the
partners are blocks 1..4.
"""

from contextlib import ExitStack

import numpy as np

FLIP = 8
B = 512
D = 128
C = 400
N = 4096
ALPHA = 0.03
E10 = float(np.exp(np.float32(10.0)))
NJ = 5  # partner blocks per core (distances 0..4)

_CACHE = {}


def _build_nc(ebufs=8, sbufs=8, pbufs=4, gbufs=4, cbufs=2):
    import concourse.tile as tile
    from concourse import bacc, mybir

    f32 = mybir.dt.float32
    bf16 = mybir.dt.bfloat16
    AF = mybir.ActivationFunctionType
    OP = mybir.AluOpType
    AX = mybir.AxisListType

    nc = bacc.Bacc("TRN2", target_bir_lowering=False, debug=False)

    ft_d = nc.dram_tensor("ft", [D, NJ * B], bf16, kind="ExternalInput")
    fr_d = nc.dram_tensor("fr", [4, 128, NJ, D], bf16, kind="ExternalInput")
    pred_d = nc.dram_tensor("pred", [B, C], f32, kind="ExternalInput")
    lab_d = nc.dram_tensor("lab", [B, 1], f32, kind="ExternalInput")
    iota_d = nc.dram_tensor("iota", [128, C], f32, kind="ExternalInput")
    eye_d = nc.dram_tensor("eye40", [128, 128], f32, kind="ExternalInput")
    m1_d = nc.dram_tensor("m1", [128, 20], f32, kind="ExternalOutput")
    m2_d = nc.dram_tensor("m2", [128, 20], f32, kind="ExternalOutput")
    dt_d = nc.dram_tensor("dt", [128, 20], f32, kind="ExternalOutput")
    cs1_d = nc.dram_tensor("cs1", [NJ, B], f32, kind="ExternalOutput")
    cs2_d = nc.dram_tensor("cs2", [NJ, B], f32, kind="ExternalOutput")
    ce_d = nc.dram_tensor("ce", [128, 8], f32, kind="ExternalOutput")

    with tile.TileContext(nc) as tc, ExitStack() as ctx:
        const = ctx.enter_context(tc.tile_pool(name="const", bufs=1))
        gpool = ctx.enter_context(tc.tile_pool(name="gp", bufs=gbufs, space="PSUM"))
        cpool = ctx.enter_context(tc.tile_pool(name="cp", bufs=cbufs, space="PSUM"))
        epool = ctx.enter_context(tc.tile_pool(name="ep", bufs=ebufs))
        spool = ctx.enter_context(tc.tile_pool(name="sp", bufs=sbufs))
        fpool = ctx.enter_context(tc.tile_pool(name="fp", bufs=2))
        ppool = ctx.enter_context(tc.tile_pool(name="pp", bufs=pbufs))
        small = ctx.enter_context(tc.tile_pool(name="sm", bufs=1))

        ftt = const.tile([D, NJ * B], bf16)
        iott = const.tile([128, C], f32)
        eyet = const.tile([128, 128], f32)
        ones = const.tile([128, 1], bf16)
        nc.vector.memset(ones[:], 1.0)
        M1 = small.tile([128, 20], f32)
        M2 = small.tile([128, 20], f32)
        dte = small.tile([128, 20], f32)
        cet = small.tile([128, 8], f32)

        # All input DMAs up front, hand-ordered: first ft block 0 (unblocks
        # the Gram pipeline), then the CE inputs, then the rest.
        pred4 = const.tile([128, 4, C], f32)
        lab4 = const.tile([128, 4], f32)
        fr_all = const.tile([128, 4, NJ, D], bf16)
        nc.sync.dma_start(ftt[:, 0:B], ft_d[:, 0:B])
        nc.sync.dma_start(lab4[:], lab_d[:, :].rearrange("(c p) k -> p (c k)", p=128))
        nc.sync.dma_start(iott[:], iota_d[:, :])
        nc.sync.dma_start(pred4[:], pred_d[:, :].rearrange("(c p) k -> p c k", p=128))
        nc.sync.dma_start(eyet[:], eye_d[:, :])
        for j in range(1, NJ):
            nc.sync.dma_start(ftt[:, j * B : (j + 1) * B], ft_d[:, j * B : (j + 1) * B])
        nc.sync.dma_start(fr_all[:], fr_d[:, :, :, :].rearrange("r p j k -> p r j k"))

        # ---- CE loss rowsums (this core's 512 rows of predicts) ----
        for c in range(4):
            mask = ppool.tile([128, C], f32)
            nc.vector.tensor_scalar(
                mask[:], iott[:], lab4[:, c : c + 1], None, OP.is_equal
            )
            scr = ppool.tile([128, C], f32)
            nc.vector.scalar_tensor_tensor(
                scr[:], mask[:], 1.0, pred4[:, c, :], OP.mult, OP.mult,
                accum_out=cet[:, 4 + c : 5 + c],
            )
            scr2 = ppool.tile([128, C], f32)
            nc.scalar.activation(
                scr2[:], pred4[:, c, :], AF.Exp, bias=0.0, scale=1.0,
                accum_out=cet[:, c : c + 1],
            )

        # ---- d_ab[p] = f_a[row] . f_b[row], all partners at once ----
        dvw = dte[:].rearrange("p (j r) -> p j r", r=4)
        for r in range(4):
            frt = fr_all[:, r, :, :]
            prod = fpool.tile([128, NJ, D], f32)
            nc.gpsimd.tensor_tensor(
                prod[:], frt, frt[:, 0:1, :].to_broadcast([128, NJ, D]), OP.mult
            )
            nc.vector.tensor_reduce(
                dvw[:, :, r], prod[:], axis=AX.X, op=OP.add
            )

        # ---- Gram blocks + moments (j outer so colsum PSUM accums are
        #      only live within one j iteration) ----
        for j in range(NJ):
            need_cs = j in (1, 2, 3)
            if need_cs:
                cs1t = cpool.tile([1, B], f32, tag="cs1t")
                cs2t = cpool.tile([1, B], f32, tag="cs2t")
            for r in range(4):
                cidx = j * 4 + r
                gt = gpool.tile([128, B], f32)
                nc.tensor.matmul(
                    gt[:],
                    ftt[:, r * 128 : (r + 1) * 128],
                    ftt[:, j * B : (j + 1) * B],
                    start=True,
                    stop=True,
                )
                if j == 0:
                    # own-block diag: g_pp(=1) -> g_pp-40 so exp(k*10*g)=0
                    nc.vector.tensor_sub(
                        gt[:, r * 128 : (r + 1) * 128],
                        gt[:, r * 128 : (r + 1) * 128],
                        eyet[:],
                    )
                et = epool.tile([128, B], bf16)
                nc.scalar.activation(
                    et[:], gt[:], AF.Exp, bias=0.0, scale=10.0,
                    accum_out=M1[:, cidx : cidx + 1],
                )
                e2 = spool.tile([128, B], bf16)
                nc.vector.scalar_tensor_tensor(
                    e2[:], et[:], 1.0, et[:], OP.mult, OP.mult,
                    accum_out=M2[:, cidx : cidx + 1],
                )
                if need_cs:
                    nc.tensor.matmul(
                        cs1t[:], ones[:], et[:],
                        start=(r == 0), stop=(r == 3),
                    )
                    nc.tensor.matmul(
                        cs2t[:], ones[:], e2[:],
                        start=(r == 0), stop=(r == 3),
                    )
            if need_cs:
                # PSUM can't DMA directly; stage via SBUF (ACT/DVE split)
                cs1s = spool.tile([1, B], f32, tag="cs1s")
                nc.scalar.copy(cs1s[:], cs1t[:])
                nc.sync.dma_start(cs1_d[j : j + 1, :], cs1s[:])
                cs2s = spool.tile([1, B], f32, tag="cs2s")
                nc.vector.tensor_copy(cs2s[:], cs2t[:])
                nc.sync.dma_start(cs2_d[j : j + 1, :], cs2s[:])

        nc.sync.dma_start(m1_d[:, :], M1[:])
        nc.sync.dma_start(m2_d[:, :], M2[:])
        nc.sync.dma_start(dt_d[:, :], dte[:])
        nc.sync.dma_start(ce_d[:, :], cet[:])

    nc.compile()
    return nc


def _get_nc(**kw):
    key = tuple(sorted(kw.items()))
    if key not in _CACHE:
        _CACHE[key] = _build_nc(**kw)
    return _CACHE[key]


def _prep_in_maps(predicts, labels, features):
    import ml_dtypes

    feats = np.ascontiguousarray(features, dtype=np.float32)
    pred = np.ascontiguousarray(predicts, dtype=np.float32)
    lab = np.asarray(labels).astype(np.float32).reshape(N, 1)
    f8 = feats.reshape(B, FLIP, D).transpose(1, 0, 2)  # [8,512,128], f8[c]=feats[c::8]
    iota = np.ascontiguousarray(
        np.broadcast_to(np.arange(C, dtype=np.float32), (128, C))
    )
    eye40 = (40.0 * np.eye(128)).astype(np.float32)
    in_maps = []
    for a in range(FLIP):
        order = [(a + i) % FLIP for i in range(NJ)]
        fo = f8[order]  # [5, 512, 128]: own class then distance 1..4 partners
        ft = np.ascontiguousarray(fo.transpose(2, 0, 1).reshape(D, NJ * B)).astype(
            ml_dtypes.bfloat16
        )
        fr = np.ascontiguousarray(
            fo.transpose(1, 0, 2).reshape(4, 128, NJ, D)
        ).astype(ml_dtypes.bfloat16)
        in_maps.append(
            {
                "ft": ft,
                "fr": fr,
                "pred": np.ascontiguousarray(pred[a * B : (a + 1) * B]),
                "lab": np.ascontiguousarray(lab[a * B : (a + 1) * B]),
                "iota": iota,
                "eye40": eye40,
            }
        )
    return in_maps


def _combine(outs):
    """Host-side O(rows) combine: reroute per-block sums between the
    ordered halves, apply the closed-form series, sum the partials."""
    S1 = {}
    S2 = {}
    dv = {}
    for c in range(FLIP):
        m1 = np.asarray(outs[c]["m1"], np.float64)  # [128, 20] cols j*4+r
        m2 = np.asarray(outs[c]["m2"], np.float64)
        dt = np.asarray(outs[c]["dt"], np.float64)
        cs1 = np.asarray(outs[c]["cs1"], np.float64)  # [5, 512], rows 1..3 used
        cs2 = np.asarray(outs[c]["cs2"], np.float64)
        for j in range(NJ):
            b = (c + j) % FLIP
            # rowsum vectors over rows of f_c: chunk r -> rows 128r..128r+127
            S1[(c, b)] = m1[:, j * 4 : (j + 1) * 4].T.reshape(B)
            S2[(c, b)] = m2[:, j * 4 : (j + 1) * 4].T.reshape(B)
            dv[(c, b)] = dt[:, j * 4 : (j + 1) * 4].T.reshape(B)
            dv[(b, c)] = dv[(c, b)]  # d is batch-indexed, symmetric in (a,b)
        for j in (1, 2, 3):
            # colsum of block (c, c+j) = rowsum of block (c+j, c)
            S1[((c + j) % FLIP, c)] = cs1[j]
            S2[((c + j) % FLIP, c)] = cs2[j]

    nce = 0.0
    for a in range(FLIP):
        S10 = S1[(a, a)]  # diag-zeroed own-block rowsum
        S20 = S2[(a, a)]
        for b in range(FLIP):
            d = dv[(a, b)]
            if a == b:
                N1 = 2.0 * S10
                N2 = 2.0 * S20
                Dv = N1 + E10
                R = 1.0 / Dv
                half = 10.0 * d - np.log(Dv) - (N1 * R + 0.5 * N2 * R * R)
                nce += 2.0 * half.sum()
            else:
                N1 = S10 + S1[(a, b)]
                N2 = S20 + S2[(a, b)]
                Dv = N1
                R = 1.0 / Dv
                half = (
                    10.0 * d
                    - np.log(Dv)
                    - (N1 * R + 0.5 * N2 * R * R)
                    - np.log1p(-np.exp(10.0 * d) * R)
                )
                nce += half.sum()

    ce = 0.0
    for c in range(FLIP):
        se = np.asarray(outs[c]["ce"], np.float64)[:, 0:4]
        xlab = np.asarray(outs[c]["ce"], np.float64)[:, 4:8]
        ce += (np.log(se) - xlab).sum()
    val = ALPHA * (-(nce) / 1024.0) + ce / N
    return np.array(val, dtype=np.float32)


def _run_hw(in_maps, trace=False):
    from concourse.bass_utils import run_bass_kernel_spmd

    nc = _get_nc()
    res = run_bass_kernel_spmd(nc, in_maps, core_ids=list(range(FLIP)), trace=trace)
    return res


def kernel(predicts, labels, features, indexs=None, **_):
    in_maps = _prep_in_maps(predicts, labels, features)
    res = _run_hw(in_maps)
    return _combine(res.results)


def kernel_sim(predicts, labels, features, indexs=None, **_):
    """CoreSim (CPU simulator) path for fast correctness iteration."""
    from concourse.bass_interp import CoreSim

    nc = _get_nc()
    in_maps = _prep_in_maps(predicts, labels, features)
    outs = []
    for a in range(FLIP):
        sim = CoreSim(nc, trace=False)
        for k, v in in_maps[a].items():
            sim.tensor(k)[:] = v
        sim.simulate()
        outs.append(
            {
                k: np.array(sim.tensor(k))
                for k in ("m1", "m2", "dt", "cs1", "cs2", "ce")
            }
        )
    return _combine(outs)



# revision 14
# speedup vs baseline: 1.6210x; 1.6210x over previous
"""Trainium2 Bass kernel for nn_BatchFlipLoss (NCE batch-flip loss + CE loss).

Math reformulation (first-order series; validated rel-err ~1e-4 vs the
jax reference, tolerance 2e-2):

The reference sums BatchCriterion over 36 flip-class pairs (i,j), j>=i.
For pair (i,j) with x = [f_i; f_j] (f_c = features[c::8], L2-normalized,
B=512 rows each), T=0.1, the loss decomposes over ordered halves (a,b).
With E_ab = exp(10*G_ab), G_ab = f_a@f_b.T, S_ab = rowsum(E_ab),
dg_ab[p] = E_ab[p,p] = exp(10 f_a[p].f_b[p]):

  a != b:  D = S0_aa + S_ab (S0_aa diag-removed own-block rowsum)
           half = ln(dg) - ln(D) - 1 - ln(1 - dg/D)
           [ln(1-x) ~ -x to first order; sum_q x_q = 1. The dropped
            x^2/2 term contributes ~9e-5 relative after alpha/1024.]
  a == b:  N1 = 2*S0_aa, D = N1 + e^10
           half = 10 - ln(D) - N1/D, pair = 2*sum(half)
           (the cross-diag term cancels -ln(1-pmt) exactly)

Per-core device work (SPMD, core c handles blocks (c, c+j mod 8),
j=0..4; distance-4 blocks are computed on both endpoints):
  - Gram: 20 bf16 matmuls [128x128]x[128,512] into 2-bank PSUM tiles
    [128,1024] (j-major: lhsT = own-block row chunks, rhs = partner).
  - One Exp activation per [128,1024] tile (10 total) -> bf16 E tiles.
  - Rowsums S: per-512-slice DVE tensor_scalar (mult,add) accum_out
    (runs in the 4x DVE perf mode).
  - Diagonals dg: Pool (GPSIMD) stt against a bf16 identity with
    accum_out — gives exp(10 d) exactly as summed into S, so the host
    subtracts it exactly for S0_aa and takes ln(dg) for the d terms.
  - Colsums (j=1,2,3, the reverse-direction rowsums the partner core
    needs): ones-lhsT matmuls accumulated over the four row chunks into
    partitions 0/32/64 of a single PSUM bank; one copy stages it out.
  - CE: one Exp over bf16 predicts [128,1600] + 4 DVE accum rowsums.
    (The label-logit term pred[p, lab[p]] is pure indexing; the host
    gathers it from the f32 input directly.)
The host reroutes these O(rows) vectors between cores and applies the
closed-form combine. All O(N^2) work stays on device.
"""

from contextlib import ExitStack

import numpy as np

FLIP = 8
B = 512
D = 128
C = 400
N = 4096
ALPHA = 0.03
E10 = float(np.exp(np.float64(10.0)))
NJ = 5  # partner blocks per core (distances 0..4)

_CACHE = {}


def _build_nc():
    import concourse.tile as tile
    from concourse import bacc, mybir

    f32 = mybir.dt.float32
    bf16 = mybir.dt.bfloat16
    AF = mybir.ActivationFunctionType
    OP = mybir.AluOpType

    nc = bacc.Bacc("TRN2", target_bir_lowering=False, debug=False)

    ft_d = nc.dram_tensor("ft", [D, NJ * B], bf16, kind="ExternalInput")
    pred_d = nc.dram_tensor("pred", [B, C], bf16, kind="ExternalInput")
    eye_d = nc.dram_tensor("eye", [128, 128], bf16, kind="ExternalInput")
    m1_d = nc.dram_tensor("m1", [128, 20], f32, kind="ExternalOutput")
    dg_d = nc.dram_tensor("dg", [128, 20], f32, kind="ExternalOutput")
    cs_d = nc.dram_tensor("cs", [65, B], f32, kind="ExternalOutput")
    ce_d = nc.dram_tensor("ce", [128, 4], f32, kind="ExternalOutput")

    with tile.TileContext(nc) as tc, ExitStack() as ctx:
        const = ctx.enter_context(tc.tile_pool(name="const", bufs=1))
        gpool = ctx.enter_context(tc.tile_pool(name="gp", bufs=3, space="PSUM"))
        cpool = ctx.enter_context(tc.tile_pool(name="cp", bufs=1, space="PSUM"))
        epool = ctx.enter_context(tc.tile_pool(name="ep", bufs=10))
        small = ctx.enter_context(tc.tile_pool(name="sm", bufs=1))

        ftt = const.tile([D, NJ * B], bf16)
        pred4 = const.tile([128, 4, C], bf16)
        eyet = const.tile([128, 128], bf16)
        ones = const.tile([128, 1], bf16)
        nc.vector.memset(ones[:], 1.0)

        M1 = small.tile([128, 20], f32)
        DG = small.tile([128, 20], f32)
        CET = small.tile([128, 4], f32)
        CSS = small.tile([65, B], f32)
        scrD = small.tile([128, 1024], bf16)  # DVE accum scratch output
        scrP = small.tile([128, 128], bf16)  # Pool diag scratch output
        cee = small.tile([128, 4, C], bf16)

        # Input DMAs: own block first (unblocks the Gram pipeline).
        nc.sync.dma_start(ftt[:, 0:B], ft_d[:, 0:B])
        nc.sync.dma_start(ftt[:, B : NJ * B], ft_d[:, B : NJ * B])
        nc.sync.dma_start(eyet[:], eye_d[:, :])
        nc.sync.dma_start(pred4[:], pred_d[:, :].rearrange("(c p) k -> p c k", p=128))

        cst = cpool.tile([128, B], f32)
        # zero the colsum bank up front (DVE is idle during the DMA wait);
        # the copy below then reads partitions 0..64 contiguously.
        nc.vector.memset(cst[0:65, :], 0.0)

        for j in range(NJ):
            for h in range(2):
                gt = gpool.tile([128, 1024], f32)
                for s in range(2):
                    r = 2 * h + s
                    nc.tensor.matmul(
                        gt[:, s * B : (s + 1) * B],
                        ftt[:, r * 128 : (r + 1) * 128],
                        ftt[:, j * B : (j + 1) * B],
                        start=True,
                        stop=True,
                    )
                et = epool.tile([128, 1024], bf16)
                nc.scalar.activation(et[:], gt[:], AF.Exp, bias=0.0, scale=10.0)
                for s in range(2):
                    r = 2 * h + s
                    cidx = j * 4 + r
                    nc.vector.tensor_scalar(
                        scrD[:, s * B : (s + 1) * B],
                        et[:, s * B : (s + 1) * B],
                        1.0, 0.0, OP.mult, OP.add,
                        accum_out=M1[:, cidx : cidx + 1],
                    )
                    off = s * B + 128 * r
                    nc.vector.scalar_tensor_tensor(
                        scrP[:], et[:, off : off + 128], 1.0, eyet[:],
                        OP.mult, OP.mult,
                        accum_out=DG[:, cidx : cidx + 1],
                    )
                    if j in (1, 2, 3):
                        nc.tensor.matmul(
                            cst[32 * (j - 1) : 32 * (j - 1) + 1, :],
                            ones[:],
                            et[:, s * B : (s + 1) * B],
                            start=(h == 0 and s == 0),
                            stop=(h == 1 and s == 1),
                        )
            if j == 3:
                # all colsums done; stage the PSUM bank out (rows 0/32/64)
                nc.vector.tensor_copy(CSS[:], cst[0:65, :])
                nc.sync.dma_start(cs_d[:, :], CSS[:])

        # ---- CE rowsums: one exp over all 4 row-chunks, 4 accum sums ----
        nc.scalar.activation(
            cee[:].rearrange("p c k -> p (c k)"),
            pred4[:].rearrange("p c k -> p (c k)"),
            AF.Exp, bias=0.0, scale=1.0,
        )
        for c in range(4):
            nc.vector.tensor_scalar(
                scrD[:, 0:C], cee[:, c, :], 1.0, 0.0, OP.mult, OP.add,
                accum_out=CET[:, c : c + 1],
            )

        nc.sync.dma_start(dg_d[:, :], DG[:])
        nc.sync.dma_start(m1_d[:, :], M1[:])
        nc.sync.dma_start(ce_d[:, :], CET[:])

    nc.compile()
    return nc


def _get_nc(**kw):
    key = tuple(sorted(kw.items()))
    if key not in _CACHE:
        _CACHE[key] = _build_nc(**kw)
    return _CACHE[key]


def _prep_in_maps(predicts, labels, features):
    import ml_dtypes

    feats = np.ascontiguousarray(features, dtype=np.float32)
    pred = np.ascontiguousarray(predicts, dtype=np.float32)
    f8 = feats.reshape(B, FLIP, D).transpose(1, 0, 2)  # [8,512,128], f8[c]=feats[c::8]
    eye = np.eye(128, dtype=np.float32).astype(ml_dtypes.bfloat16)
    in_maps = []
    for a in range(FLIP):
        order = [(a + i) % FLIP for i in range(NJ)]
        fo = f8[order]  # [5, 512, 128]: own class then distance 1..4 partners
        ft = np.ascontiguousarray(fo.transpose(2, 0, 1).reshape(D, NJ * B)).astype(
            ml_dtypes.bfloat16
        )
        in_maps.append(
            {
                "ft": ft,
                "pred": np.ascontiguousarray(pred[a * B : (a + 1) * B]).astype(
                    ml_dtypes.bfloat16
                ),
                "eye": eye,
            }
        )
    return in_maps


def _combine(outs, predicts, labels):
    """Host-side O(rows) combine: reroute per-block sums between the
    ordered halves, apply the first-order closed form, sum the partials."""
    S1 = {}
    dgv = {}
    for c in range(FLIP):
        m1 = np.asarray(outs[c]["m1"], np.float64)  # [128, 20] cols j*4+r
        dg = np.asarray(outs[c]["dg"], np.float64)
        cs = np.asarray(outs[c]["cs"], np.float64)  # [65, 512]; rows 0/32/64
        for j in range(NJ):
            b = (c + j) % FLIP
            # rowsum vectors over rows of f_c: chunk r -> rows 128r..128r+127
            S1[(c, b)] = m1[:, j * 4 : (j + 1) * 4].T.reshape(B)
            dgv[(c, b)] = dg[:, j * 4 : (j + 1) * 4].T.reshape(B)
            dgv[(b, c)] = dgv[(c, b)]  # E diag is symmetric in (a,b)
        for j in (1, 2, 3):
            # colsum of block (c, c+j) = rowsum of block (c+j, c)
            S1[((c + j) % FLIP, c)] = cs[32 * (j - 1)]

    nce = 0.0
    for a in range(FLIP):
        # remove the diagonal exactly as it was summed (bf16 values)
        S10 = S1[(a, a)] - dgv[(a, a)]
        for b in range(FLIP):
            if a == b:
                N1 = 2.0 * S10
                Dv = N1 + E10
                half = 10.0 - np.log(Dv) - N1 / Dv
                nce += 2.0 * half.sum()
            else:
                dg = dgv[(a, b)]
                Dv = S10 + S1[(a, b)]
                half = np.log(dg) - np.log(Dv) - 1.0 - np.log1p(-dg / Dv)
                nce += half.sum()

    # CE: device exp-rowsums; label logits gathered from the f32 input
    ce = 0.0
    for c in range(FLIP):
        se = np.asarray(outs[c]["ce"], np.float64)  # [128, 4]
        ce += np.log(se.T).sum()
    pred = np.asarray(predicts, np.float64)
    lab = np.asarray(labels).astype(np.int64)
    ce -= pred[np.arange(N), lab].sum()
    val = ALPHA * (-(nce) / 1024.0) + ce / N
    return np.array(val, dtype=np.float32)


def _run_hw(in_maps, trace=False):
    from concourse.bass_utils import run_bass_kernel_spmd

    nc = _get_nc()
    res = run_bass_kernel_spmd(nc, in_maps, core_ids=list(range(FLIP)), trace=trace)
    return res


def kernel(predicts, labels, features, indexs=None, **_):
    in_maps = _prep_in_maps(predicts, labels, features)
    res = _run_hw(in_maps)
    return _combine(res.results, predicts, labels)


def kernel_sim(predicts, labels, features, indexs=None, **_):
    """CoreSim (CPU simulator) path for fast correctness iteration."""
    from concourse.bass_interp import CoreSim

    nc = _get_nc()
    in_maps = _prep_in_maps(predicts, labels, features)
    outs = []
    for a in range(FLIP):
        sim = CoreSim(nc, trace=False)
        for k, v in in_maps[a].items():
            sim.tensor(k)[:] = v
        sim.simulate()
        outs.append(
            {k: np.array(sim.tensor(k)) for k in ("m1", "dg", "cs", "ce")}
        )
    return _combine(outs, predicts, labels)


# revision 16
# speedup vs baseline: 1.6336x; 1.0078x over previous
"""Trainium2 Bass kernel for nn_BatchFlipLoss (NCE batch-flip loss + CE loss).

Math reformulation (first-order series; validated rel-err ~1e-4 vs the
jax reference, tolerance 2e-2):

The reference sums BatchCriterion over 36 flip-class pairs (i,j), j>=i.
For pair (i,j) with x = [f_i; f_j] (f_c = features[c::8], L2-normalized,
B=512 rows each), T=0.1, the loss decomposes over ordered halves (a,b).
With E_ab = exp(10*G_ab), G_ab = f_a@f_b.T, S_ab = rowsum(E_ab),
dg_ab[p] = E_ab[p,p] = exp(10 f_a[p].f_b[p]):

  a != b:  D = S0_aa + S_ab (S0_aa diag-removed own-block rowsum)
           half = ln(dg) - ln(D) - 1 - ln(1 - dg/D)
           [ln(1-x) ~ -x to first order; sum_q x_q = 1. The dropped
            x^2/2 term contributes ~9e-5 relative after alpha/1024.]
  a == b:  N1 = 2*S0_aa, D = N1 + e^10
           half = 10 - ln(D) - N1/D, pair = 2*sum(half)
           (the cross-diag term cancels -ln(1-pmt) exactly)

Per-core device work (SPMD, core c handles blocks (c, c+j mod 8),
j=0..4; distance-4 blocks are computed on both endpoints):
  - Gram: 20 bf16 matmuls [128x128]x[128,512] into 2-bank PSUM tiles
    [128,1024] (j-major: lhsT = own-block row chunks, rhs = partner).
  - One Exp activation per [128,1024] tile (10 total) -> bf16 E tiles.
  - Rowsums S: per-512-slice DVE tensor_scalar (mult,add) accum_out
    (runs in the 4x DVE perf mode).
  - Diagonals dg: Pool (GPSIMD) stt against a bf16 identity with
    accum_out — gives exp(10 d) exactly as summed into S, so the host
    subtracts it exactly for S0_aa and takes ln(dg) for the d terms.
  - Colsums (j=1,2,3, the reverse-direction rowsums the partner core
    needs): ones-lhsT matmuls accumulated over the four row chunks into
    partitions 0/32/64 of a single PSUM bank; one copy stages it out.
  - CE: one Exp over bf16 predicts [128,1600] + 4 DVE accum rowsums.
    (The label-logit term pred[p, lab[p]] is pure indexing; the host
    gathers it from the f32 input directly.)
The host reroutes these O(rows) vectors between cores and applies the
closed-form combine. All O(N^2) work stays on device.
"""

from contextlib import ExitStack

import numpy as np

FLIP = 8
B = 512
D = 128
C = 400
N = 4096
ALPHA = 0.03
E10 = float(np.exp(np.float64(10.0)))
NJ = 5  # partner blocks per core (distances 0..4)

_CACHE = {}


def _build_nc():
    import concourse.tile as tile
    from concourse import bacc, mybir

    f32 = mybir.dt.float32
    bf16 = mybir.dt.bfloat16
    AF = mybir.ActivationFunctionType
    OP = mybir.AluOpType

    nc = bacc.Bacc("TRN2", target_bir_lowering=False, debug=False)

    ft_d = nc.dram_tensor("ft", [D, NJ * B], bf16, kind="ExternalInput")
    pred_d = nc.dram_tensor("pred", [B, C], bf16, kind="ExternalInput")
    eye_d = nc.dram_tensor("eye", [128, 128], bf16, kind="ExternalInput")
    m1_d = nc.dram_tensor("m1", [128, 20], f32, kind="ExternalOutput")
    dg_d = nc.dram_tensor("dg", [128, 20], f32, kind="ExternalOutput")
    cs_d = nc.dram_tensor("cs", [65, B], f32, kind="ExternalOutput")
    ce_d = nc.dram_tensor("ce", [128, 4], f32, kind="ExternalOutput")

    with tile.TileContext(nc) as tc, ExitStack() as ctx:
        const = ctx.enter_context(tc.tile_pool(name="const", bufs=1))
        gpool = ctx.enter_context(tc.tile_pool(name="gp", bufs=3, space="PSUM"))
        cpool = ctx.enter_context(tc.tile_pool(name="cp", bufs=1, space="PSUM"))
        epool = ctx.enter_context(tc.tile_pool(name="ep", bufs=10))
        small = ctx.enter_context(tc.tile_pool(name="sm", bufs=1))

        ftt = const.tile([D, NJ * B], bf16)
        pred4 = const.tile([128, 4, C], bf16)
        eyet = const.tile([128, 128], bf16)
        ones = const.tile([128, 1], bf16)
        nc.vector.memset(ones[:], 1.0)

        M1 = small.tile([128, 20], f32)
        DG = small.tile([128, 20], f32)
        CET = small.tile([128, 4], f32)
        CSS = small.tile([65, B], f32)
        scrD = small.tile([128, 1024], bf16)  # DVE accum scratch output
        scrP = small.tile([128, 128], bf16)  # Pool diag scratch output
        cee = small.tile([128, 4, C], bf16)

        # Input DMAs: own block first (unblocks the Gram pipeline).
        nc.sync.dma_start(ftt[:, 0:B], ft_d[:, 0:B])
        nc.sync.dma_start(ftt[:, B : NJ * B], ft_d[:, B : NJ * B])
        nc.sync.dma_start(eyet[:], eye_d[:, :])
        nc.sync.dma_start(pred4[:], pred_d[:, :].rearrange("(c p) k -> p c k", p=128))

        cst = cpool.tile([128, B], f32)
        # zero the colsum bank up front (DVE is idle during the DMA wait);
        # the copy below then reads partitions 0..64 contiguously.
        nc.vector.memset(cst[0:65, :], 0.0)

        # Tile plan per j: list of chunk groups. j=0 leads with two
        # half-width tiles so the first exp starts one matmul after the
        # ft DMA; j=4 trails with two so the final consumer chain is
        # short. CE's exp is slotted before j=4 so its accum sums finish
        # under the last gram exps.
        plans = {0: [[0], [1], [2, 3]], 4: [[0, 1], [2], [3]]}

        def gram_tile(j, rs, cs_flags):
            gt = gpool.tile([128, 1024], f32)
            w = len(rs) * B
            for s, r in enumerate(rs):
                nc.tensor.matmul(
                    gt[:, s * B : (s + 1) * B],
                    ftt[:, r * 128 : (r + 1) * 128],
                    ftt[:, j * B : (j + 1) * B],
                    start=True,
                    stop=True,
                )
            et = epool.tile([128, 1024], bf16)
            nc.scalar.activation(
                et[:, 0:w], gt[:, 0:w], AF.Exp, bias=0.0, scale=10.0
            )
            for s, r in enumerate(rs):
                cidx = j * 4 + r
                nc.vector.tensor_scalar(
                    scrD[:, s * B : (s + 1) * B],
                    et[:, s * B : (s + 1) * B],
                    1.0, 0.0, OP.mult, OP.add,
                    accum_out=M1[:, cidx : cidx + 1],
                )
                off = s * B + 128 * r
                nc.vector.scalar_tensor_tensor(
                    scrP[:], et[:, off : off + 128], 1.0, eyet[:],
                    OP.mult, OP.mult,
                    accum_out=DG[:, cidx : cidx + 1],
                )
                if j in (1, 2, 3):
                    nc.tensor.matmul(
                        cst[32 * (j - 1) : 32 * (j - 1) + 1, :],
                        ones[:],
                        et[:, s * B : (s + 1) * B],
                        start=(r == 0),
                        stop=(r == 3),
                    )

        def ce_sums():
            nc.scalar.activation(
                cee[:].rearrange("p c k -> p (c k)"),
                pred4[:].rearrange("p c k -> p (c k)"),
                AF.Exp, bias=0.0, scale=1.0,
            )
            for c in range(4):
                nc.vector.tensor_scalar(
                    scrD[:, 0:C], cee[:, c, :], 1.0, 0.0, OP.mult, OP.add,
                    accum_out=CET[:, c : c + 1],
                )

        for j in range(NJ):
            if j == 4:
                ce_sums()
                nc.sync.dma_start(ce_d[:, :], CET[:])
            for rs in plans.get(j, [[0, 1], [2, 3]]):
                gram_tile(j, rs, None)
            if j == 3:
                # all colsums done; stage the PSUM bank out (rows 0/32/64)
                nc.vector.tensor_copy(CSS[:], cst[0:65, :])
                nc.sync.dma_start(cs_d[:, :], CSS[:])

        nc.sync.dma_start(m1_d[:, :], M1[:])
        nc.sync.dma_start(dg_d[:, :], DG[:])

    nc.compile()
    return nc


def _get_nc(**kw):
    key = tuple(sorted(kw.items()))
    if key not in _CACHE:
        _CACHE[key] = _build_nc(**kw)
    return _CACHE[key]


def _prep_in_maps(predicts, labels, features):
    import ml_dtypes

    feats = np.ascontiguousarray(features, dtype=np.float32)
    pred = np.ascontiguousarray(predicts, dtype=np.float32)
    f8 = feats.reshape(B, FLIP, D).transpose(1, 0, 2)  # [8,512,128], f8[c]=feats[c::8]
    eye = np.eye(128, dtype=np.float32).astype(ml_dtypes.bfloat16)
    in_maps = []
    for a in range(FLIP):
        order = [(a + i) % FLIP for i in range(NJ)]
        fo = f8[order]  # [5, 512, 128]: own class then distance 1..4 partners
        ft = np.ascontiguousarray(fo.transpose(2, 0, 1).reshape(D, NJ * B)).astype(
            ml_dtypes.bfloat16
        )
        in_maps.append(
            {
                "ft": ft,
                "pred": np.ascontiguousarray(pred[a * B : (a + 1) * B]).astype(
                    ml_dtypes.bfloat16
                ),
                "eye": eye,
            }
        )
    return in_maps


def _combine(outs, predicts, labels):
    """Host-side O(rows) combine: reroute per-block sums between the
    ordered halves, apply the first-order closed form, sum the partials."""
    S1 = {}
    dgv = {}
    for c in range(FLIP):
        m1 = np.asarray(outs[c]["m1"], np.float64)  # [128, 20] cols j*4+r
        dg = np.asarray(outs[c]["dg"], np.float64)
        cs = np.asarray(outs[c]["cs"], np.float64)  # [65, 512]; rows 0/32/64
        for j in range(NJ):
            b = (c + j) % FLIP
            # rowsum vectors over rows of f_c: chunk r -> rows 128r..128r+127
            S1[(c, b)] = m1[:, j * 4 : (j + 1) * 4].T.reshape(B)
            dgv[(c, b)] = dg[:, j * 4 : (j + 1) * 4].T.reshape(B)
            dgv[(b, c)] = dgv[(c, b)]  # E diag is symmetric in (a,b)
        for j in (1, 2, 3):
            # colsum of block (c, c+j) = rowsum of block (c+j, c)
            S1[((c + j) % FLIP, c)] = cs[32 * (j - 1)]

    nce = 0.0
    for a in range(FLIP):
        # remove the diagonal exactly as it was summed (bf16 values)
        S10 = S1[(a, a)] - dgv[(a, a)]
        for b in range(FLIP):
            if a == b:
                N1 = 2.0 * S10
                Dv = N1 + E10
                half = 10.0 - np.log(Dv) - N1 / Dv
                nce += 2.0 * half.sum()
            else:
                dg = dgv[(a, b)]
                Dv = S10 + S1[(a, b)]
                half = np.log(dg) - np.log(Dv) - 1.0 - np.log1p(-dg / Dv)
                nce += half.sum()

    # CE: device exp-rowsums; label logits gathered from the f32 input
    ce = 0.0
    for c in range(FLIP):
        se = np.asarray(outs[c]["ce"], np.float64)  # [128, 4]
        ce += np.log(se.T).sum()
    pred = np.asarray(predicts, np.float64)
    lab = np.asarray(labels).astype(np.int64)
    ce -= pred[np.arange(N), lab].sum()
    val = ALPHA * (-(nce) / 1024.0) + ce / N
    return np.array(val, dtype=np.float32)


def _run_hw(in_maps, trace=False):
    from concourse.bass_utils import run_bass_kernel_spmd

    nc = _get_nc()
    res = run_bass_kernel_spmd(nc, in_maps, core_ids=list(range(FLIP)), trace=trace)
    return res


def kernel(predicts, labels, features, indexs=None, **_):
    in_maps = _prep_in_maps(predicts, labels, features)
    res = _run_hw(in_maps)
    return _combine(res.results, predicts, labels)


def kernel_sim(predicts, labels, features, indexs=None, **_):
    """CoreSim (CPU simulator) path for fast correctness iteration."""
    from concourse.bass_interp import CoreSim

    nc = _get_nc()
    in_maps = _prep_in_maps(predicts, labels, features)
    outs = []
    for a in range(FLIP):
        sim = CoreSim(nc, trace=False)
        for k, v in in_maps[a].items():
            sim.tensor(k)[:] = v
        sim.simulate()
        outs.append(
            {k: np.array(sim.tensor(k)) for k in ("m1", "dg", "cs", "ce")}
        )
    return _combine(outs, predicts, labels)
